# revision 5
# baseline (speedup 1.0000x reference)
import numpy as np
from itertools import combinations

V = 3000
NCORES = 8
VC = V // NCORES          # 375 vertices per core
P = 128
NB = 3                    # blocks of 128 partitions per core
VPAD = NB * P             # 384
T = 56                    # triangles = C(8,3)
RA = 40                   # template points (5*8)
NN = 8                    # neighbors
F_IN = 488
F_OUT = 160
BIG = 1.0e30

TRI = np.array(list(combinations(range(NN), 3)), dtype=np.int64)  # (56,3) lex

# packed input offsets
oPX, oPY, oTX, oTY = 0, 8, 16, 56
oAX, oAY, oBX, oBY, oCX, oCY, oCD = 96, 152, 208, 264, 320, 376, 432


def _runs():
    i_runs, ij_runs = [], []
    t = 0
    while t < T:
        i = TRI[t, 0]
        t0 = t
        while t < T and TRI[t, 0] == i:
            t += 1
        i_runs.append((int(i), t0, t - t0))
    t = 0
    while t < T:
        i, j = TRI[t, 0], TRI[t, 1]
        t0 = t
        while t < T and TRI[t, 0] == i and TRI[t, 1] == j:
            t += 1
        ij_runs.append((int(i), int(j), t0, t - t0))
    return i_runs, ij_runs


def _build():
    from concourse import bacc, tile
    import concourse.mybir as mybir

    f32 = mybir.dt.float32
    Alu = mybir.AluOpType
    ActF = mybir.ActivationFunctionType
    AxL = mybir.AxisListType

    nc = bacc.Bacc(None, target_bir_lowering=False)
    x = nc.dram_tensor("x", [VPAD, F_IN], f32, kind="ExternalInput")
    out = nc.dram_tensor("out", [VPAD, F_OUT], f32, kind="ExternalOutput")
    i_runs, ij_runs = _runs()

    def bt(ap, n):  # (128, m) / (128, a, b) -> broadcast new LAST dim of n
        return ap.unsqueeze(len(ap.shape)).broadcast_to([*ap.shape, n])

    def bm(ap, m):  # (128, n) -> (128, m, n)
        return ap.unsqueeze(1).broadcast_to([P, m, ap.shape[1]])

    with tile.TileContext(nc) as tc:
        with tc.tile_pool(name="io", bufs=2) as io, \
             tc.tile_pool(name="sm", bufs=1) as sm, \
             tc.tile_pool(name="md", bufs=1) as md, \
             tc.tile_pool(name="bg", bufs=1) as bg:
            for b in range(NB):
                xt = io.tile([P, F_IN], f32, name="xt", tag="xt")
                nc.sync.dma_start(xt[:, :], x[b * P:(b + 1) * P, :])
                PX = xt[:, oPX:oPX + NN]
                PY = xt[:, oPY:oPY + NN]
                TX = xt[:, oTX:oTX + RA]
                TY = xt[:, oTY:oTY + RA]
                AX = xt[:, oAX:oAX + T]
                AY = xt[:, oAY:oAY + T]
                BX = xt[:, oBX:oBX + T]
                BY = xt[:, oBY:oBY + T]
                CX = xt[:, oCX:oCX + T]
                CY = xt[:, oCY:oCY + T]
                CD = xt[:, oCD:oCD + T]

                def s56(tag):
                    return sm.tile([P, T], f32, name=tag, tag=tag)

                # ---- per-triangle (56) edge vectors & dots ----
                v0x, v0y, v1x, v1y = s56("v0x"), s56("v0y"), s56("v1x"), s56("v1y")
                nc.vector.tensor_tensor(v0x[:, :], CX, AX, op=Alu.subtract)
                nc.vector.tensor_tensor(v0y[:, :], CY, AY, op=Alu.subtract)
                nc.vector.tensor_tensor(v1x[:, :], BX, AX, op=Alu.subtract)
                nc.vector.tensor_tensor(v1y[:, :], BY, AY, op=Alu.subtract)
                ta, tb = s56("ta"), s56("tb")
                d00, d01, d11, den, rden, s_o = (s56("d00"), s56("d01"),
                                                 s56("d11"), s56("den"),
                                                 s56("rden"), s56("s_o"))
                nc.vector.tensor_tensor(ta[:, :], v0x[:, :], v0x[:, :], op=Alu.mult)
                nc.vector.tensor_tensor(tb[:, :], v0y[:, :], v0y[:, :], op=Alu.mult)
                nc.vector.tensor_tensor(d00[:, :], ta[:, :], tb[:, :], op=Alu.add)
                nc.vector.tensor_tensor(ta[:, :], v0x[:, :], v1x[:, :], op=Alu.mult)
                nc.vector.tensor_tensor(tb[:, :], v0y[:, :], v1y[:, :], op=Alu.mult)
                nc.vector.tensor_tensor(d01[:, :], ta[:, :], tb[:, :], op=Alu.add)
                nc.vector.tensor_tensor(ta[:, :], v1x[:, :], v1x[:, :], op=Alu.mult)
                nc.vector.tensor_tensor(tb[:, :], v1y[:, :], v1y[:, :], op=Alu.mult)
                nc.vector.tensor_tensor(d11[:, :], ta[:, :], tb[:, :], op=Alu.add)
                nc.vector.tensor_tensor(ta[:, :], d00[:, :], d11[:, :], op=Alu.mult)
                nc.vector.tensor_tensor(tb[:, :], d01[:, :], d01[:, :], op=Alu.mult)
                nc.vector.tensor_tensor(den[:, :], ta[:, :], tb[:, :], op=Alu.subtract)
                nc.vector.reciprocal(rden[:, :], den[:, :])
                # orientation s = cross(B-A, C-A) = v1x*v0y - v1y*v0x
                nc.vector.tensor_tensor(ta[:, :], v1x[:, :], v0y[:, :], op=Alu.mult)
                nc.vector.tensor_tensor(tb[:, :], v1y[:, :], v0x[:, :], op=Alu.mult)
                nc.vector.tensor_tensor(s_o[:, :], ta[:, :], tb[:, :], op=Alu.subtract)

                # ---- affine coefficients for w1, w2 ----
                # w2 = (d11*dot02 - d01*dot12)*rden = a2*Tx + b2*Ty + c2
                # w1 = (d00*dot12 - d01*dot02)*rden = a1*Tx + b1*Ty + c1
                a2, b2, c2 = s56("a2"), s56("b2"), s56("c2")
                a1, b1, c1 = s56("a1"), s56("b1"), s56("c1")
                nc.vector.tensor_tensor(ta[:, :], d11[:, :], v0x[:, :], op=Alu.mult)
                nc.vector.tensor_tensor(tb[:, :], d01[:, :], v1x[:, :], op=Alu.mult)
                nc.vector.tensor_tensor(a2[:, :], ta[:, :], tb[:, :], op=Alu.subtract)
                nc.vector.tensor_tensor(a2[:, :], a2[:, :], rden[:, :], op=Alu.mult)
                nc.vector.tensor_tensor(ta[:, :], d11[:, :], v0y[:, :], op=Alu.mult)
                nc.vector.tensor_tensor(tb[:, :], d01[:, :], v1y[:, :], op=Alu.mult)
                nc.vector.tensor_tensor(b2[:, :], ta[:, :], tb[:, :], op=Alu.subtract)
                nc.vector.tensor_tensor(b2[:, :], b2[:, :], rden[:, :], op=Alu.mult)
                nc.vector.tensor_tensor(ta[:, :], a2[:, :], AX, op=Alu.mult)
                nc.vector.tensor_tensor(tb[:, :], b2[:, :], AY, op=Alu.mult)
                nc.vector.scalar_tensor_tensor(c2[:, :], ta[:, :], -1.0, tb[:, :],
                                               op0=Alu.mult, op1=Alu.subtract)
                nc.vector.tensor_tensor(ta[:, :], d00[:, :], v1x[:, :], op=Alu.mult)
                nc.vector.tensor_tensor(tb[:, :], d01[:, :], v0x[:, :], op=Alu.mult)
                nc.vector.tensor_tensor(a1[:, :], ta[:, :], tb[:, :], op=Alu.subtract)
                nc.vector.tensor_tensor(a1[:, :], a1[:, :], rden[:, :], op=Alu.mult)
                nc.vector.tensor_tensor(ta[:, :], d00[:, :], v1y[:, :], op=Alu.mult)
                nc.vector.tensor_tensor(tb[:, :], d01[:, :], v0y[:, :], op=Alu.mult)
                nc.vector.tensor_tensor(b1[:, :], ta[:, :], tb[:, :], op=Alu.subtract)
                nc.vector.tensor_tensor(b1[:, :], b1[:, :], rden[:, :], op=Alu.mult)
                nc.vector.tensor_tensor(ta[:, :], a1[:, :], AX, op=Alu.mult)
                nc.vector.tensor_tensor(tb[:, :], b1[:, :], AY, op=Alu.mult)
                nc.vector.scalar_tensor_tensor(c1[:, :], ta[:, :], -1.0, tb[:, :],
                                               op0=Alu.mult, op1=Alu.subtract)

                # ---- incircle / Delaunay on gpsimd, grid (P, T, NN) ----
                def g8(tag):
                    return md.tile([P, T, NN], f32, name=tag, tag=tag)

                iax, iay, ibx, iby, icx, icy = (g8("iax"), g8("iay"), g8("ibx"),
                                                g8("iby"), g8("icx"), g8("icy"))
                PXb = bm(PX, T)
                PYb = bm(PY, T)
                nc.gpsimd.tensor_tensor(iax[:, :, :], bt(AX, NN), PXb, op=Alu.subtract)
                nc.gpsimd.tensor_tensor(iay[:, :, :], bt(AY, NN), PYb, op=Alu.subtract)
                nc.gpsimd.tensor_tensor(ibx[:, :, :], bt(BX, NN), PXb, op=Alu.subtract)
                nc.gpsimd.tensor_tensor(iby[:, :, :], bt(BY, NN), PYb, op=Alu.subtract)
                nc.gpsimd.tensor_tensor(icx[:, :, :], bt(CX, NN), PXb, op=Alu.subtract)
                nc.gpsimd.tensor_tensor(icy[:, :, :], bt(CY, NN), PYb, op=Alu.subtract)
                iaz, ibz, icz, g1, g2 = (g8("iaz"), g8("ibz"), g8("icz"),
                                         g8("g1"), g8("g2"))
                nc.gpsimd.tensor_tensor(g1[:, :, :], iax[:, :, :], iax[:, :, :], op=Alu.mult)
                nc.gpsimd.tensor_tensor(g2[:, :, :], iay[:, :, :], iay[:, :, :], op=Alu.mult)
                nc.gpsimd.tensor_tensor(iaz[:, :, :], g1[:, :, :], g2[:, :, :], op=Alu.add)
                nc.gpsimd.tensor_tensor(g1[:, :, :], ibx[:, :, :], ibx[:, :, :], op=Alu.mult)
                nc.gpsimd.tensor_tensor(g2[:, :, :], iby[:, :, :], iby[:, :, :], op=Alu.mult)
                nc.gpsimd.tensor_tensor(ibz[:, :, :], g1[:, :, :], g2[:, :, :], op=Alu.add)
                nc.gpsimd.tensor_tensor(g1[:, :, :], icx[:, :, :], icx[:, :, :], op=Alu.mult)
                nc.gpsimd.tensor_tensor(g2[:, :, :], icy[:, :, :], icy[:, :, :], op=Alu.mult)
                nc.gpsimd.tensor_tensor(icz[:, :, :], g1[:, :, :], g2[:, :, :], op=Alu.add)
                # D = iax*(iby*icz - ibz*icy) + iay*(ibz*icx - ibx*icz)
                #     + iaz*(ibx*icy - iby*icx)
                m1, m2, m3, Dd = g8("m1"), g8("m2"), g8("m3"), g8("Dd")
                nc.gpsimd.tensor_tensor(g1[:, :, :], iby[:, :, :], icz[:, :, :], op=Alu.mult)
                nc.gpsimd.tensor_tensor(g2[:, :, :], ibz[:, :, :], icy[:, :, :], op=Alu.mult)
                nc.gpsimd.tensor_tensor(m1[:, :, :], g1[:, :, :], g2[:, :, :], op=Alu.subtract)
                nc.gpsimd.tensor_tensor(g1[:, :, :], ibz[:, :, :], icx[:, :, :], op=Alu.mult)
                nc.gpsimd.tensor_tensor(g2[:, :, :], ibx[:, :, :], icz[:, :, :], op=Alu.mult)
                nc.gpsimd.tensor_tensor(m2[:, :, :], g1[:, :, :], g2[:, :, :], op=Alu.subtract)
                nc.gpsimd.tensor_tensor(g1[:, :, :], ibx[:, :, :], icy[:, :, :], op=Alu.mult)
                nc.gpsimd.tensor_tensor(g2[:, :, :], iby[:, :, :], icx[:, :, :], op=Alu.mult)
                nc.gpsimd.tensor_tensor(m3[:, :, :], g1[:, :, :], g2[:, :, :], op=Alu.subtract)
                nc.gpsimd.tensor_tensor(g1[:, :, :], iax[:, :, :], m1[:, :, :], op=Alu.mult)
                nc.gpsimd.tensor_tensor(g2[:, :, :], iay[:, :, :], m2[:, :, :], op=Alu.mult)
                nc.gpsimd.tensor_tensor(Dd[:, :, :], g1[:, :, :], g2[:, :, :], op=Alu.add)
                nc.gpsimd.tensor_tensor(g1[:, :, :], iaz[:, :, :], m3[:, :, :], op=Alu.mult)
                nc.gpsimd.tensor_tensor(Dd[:, :, :], Dd[:, :, :], g1[:, :, :], op=Alu.add)
                # violation iff s*D > 0
                nc.gpsimd.tensor_tensor(g1[:, :, :], Dd[:, :, :], bt(s_o[:, :], NN),
                                        op=Alu.mult)
                nc.gpsimd.tensor_scalar(g2[:, :, :], g1[:, :, :], 0.0, None,
                                        op0=Alu.is_gt)
                cnt, notD = s56("cnt"), s56("notD")
                nc.vector.tensor_reduce(cnt[:, :], g2[:, :, :], axis=AxL.X, op=Alu.add)
                nc.vector.tensor_scalar(notD[:, :], cnt[:, :], 0.0, None, op0=Alu.is_le)

                # ---- point distances ed (P, RA, NN) ----
                def gra8(tag):
                    return md.tile([P, RA, NN], f32, name=tag, tag=tag)

                dex, dey, ed = gra8("dex"), gra8("dey"), gra8("ed")
                nc.vector.tensor_tensor(dex[:, :, :], bt(TX, NN), bm(PX, RA), op=Alu.subtract)
                nc.vector.tensor_tensor(dey[:, :, :], bt(TY, NN), bm(PY, RA), op=Alu.subtract)
                nc.vector.tensor_tensor(dex[:, :, :], dex[:, :, :], dex[:, :, :], op=Alu.mult)
                nc.vector.tensor_tensor(dey[:, :, :], dey[:, :, :], dey[:, :, :], op=Alu.mult)
                nc.vector.tensor_tensor(dex[:, :, :], dex[:, :, :], dey[:, :, :], op=Alu.add)
                nc.scalar.activation(ed[:, :, :], dex[:, :, :], func=ActF.Sqrt)

                # ---- big grid (P, RA, T) ----
                def big3(tag):
                    return bg.tile([P, RA, T], f32, name=tag, tag=tag)

                w1, w2, w0 = big3("w1"), big3("w2"), big3("w0")
                u1, u2 = big3("u1"), big3("u2")
                TXb = bt(TX, T)
                TYb = bt(TY, T)
                nc.vector.tensor_tensor(u1[:, :, :], bm(a2[:, :], RA), TXb, op=Alu.mult)
                nc.vector.tensor_tensor(u2[:, :, :], bm(b2[:, :], RA), TYb, op=Alu.mult)
                nc.vector.tensor_tensor(w2[:, :, :], u1[:, :, :], u2[:, :, :], op=Alu.add)
                nc.vector.tensor_tensor(w2[:, :, :], w2[:, :, :], bm(c2[:, :], RA), op=Alu.add)
                nc.vector.tensor_tensor(u1[:, :, :], bm(a1[:, :], RA), TXb, op=Alu.mult)
                nc.vector.tensor_tensor(u2[:, :, :], bm(b1[:, :], RA), TYb, op=Alu.mult)
                nc.vector.tensor_tensor(w1[:, :, :], u1[:, :, :], u2[:, :, :], op=Alu.add)
                nc.vector.tensor_tensor(w1[:, :, :], w1[:, :, :], bm(c1[:, :], RA), op=Alu.add)
                nc.vector.tensor_tensor(u1[:, :, :], w1[:, :, :], w2[:, :, :], op=Alu.add)
                nc.vector.tensor_scalar(w0[:, :, :], u1[:, :, :], -1.0, 1.0,
                                        op0=Alu.mult, op1=Alu.add)
                # inside test
                mn, mx = u1, u2
                nc.vector.tensor_tensor(mn[:, :, :], w0[:, :, :], w1[:, :, :], op=Alu.min)
                nc.vector.tensor_tensor(mn[:, :, :], mn[:, :, :], w2[:, :, :], op=Alu.min)
                nc.vector.tensor_tensor(mx[:, :, :], w0[:, :, :], w1[:, :, :], op=Alu.max)
                nc.vector.tensor_tensor(mx[:, :, :], mx[:, :, :], w2[:, :, :], op=Alu.max)
                good = big3("good")
                nc.vector.tensor_scalar(mn[:, :, :], mn[:, :, :], 0.0, None, op0=Alu.is_gt)
                nc.vector.tensor_scalar(mx[:, :, :], mx[:, :, :], 1.0, None, op0=Alu.is_lt)
                nc.vector.tensor_tensor(good[:, :, :], mn[:, :, :], mx[:, :, :], op=Alu.mult)
                nc.vector.tensor_tensor(good[:, :, :], good[:, :, :], bm(notD[:, :], RA),
                                        op=Alu.mult)

                # ---- dist assembly from ed runs ----
                dist = big3("dist")
                for (i, t0, L) in i_runs:
                    nc.vector.tensor_copy(
                        dist[:, :, t0:t0 + L],
                        ed[:, :, i].unsqueeze(2).broadcast_to([P, RA, L]))
                for (i, j, t0, L) in ij_runs:
                    nc.vector.tensor_tensor(
                        dist[:, :, t0:t0 + L], dist[:, :, t0:t0 + L],
                        ed[:, :, j].unsqueeze(2).broadcast_to([P, RA, L]), op=Alu.add)
                for (i, j, t0, L) in ij_runs:
                    nc.vector.tensor_tensor(
                        dist[:, :, t0:t0 + L], dist[:, :, t0:t0 + L],
                        ed[:, :, j + 1:j + 1 + L], op=Alu.add)

                # ---- masked argmin & selection ----
                dm = big3("dm")
                nc.vector.memset(dm[:, :, :], BIG)
                nc.vector.copy_predicated(dm[:, :, :], good[:, :, :].bitcast(mybir.dt.int32), dist[:, :, :])
                ot = io.tile([P, F_OUT], f32, name="ot", tag="ot")
                m40 = md.tile([P, RA], f32, name="m40", tag="m40")
                nc.vector.tensor_reduce(m40[:, :], dm[:, :, :], axis=AxL.X, op=Alu.min)
                maskm = dist  # reuse buffer
                nc.vector.tensor_tensor(maskm[:, :, :], dm[:, :, :], bt(m40[:, :], T),
                                        op=Alu.is_equal)
                sel = dm  # reuse buffer
                nc.vector.memset(sel[:, :, :], 0.0)
                nc.vector.copy_predicated(sel[:, :, :], maskm[:, :, :].bitcast(mybir.dt.int32), w1[:, :, :])
                nc.vector.tensor_reduce(ot[:, 40:80], sel[:, :, :], axis=AxL.X, op=Alu.add)
                nc.vector.memset(sel[:, :, :], 0.0)
                nc.vector.copy_predicated(sel[:, :, :], maskm[:, :, :].bitcast(mybir.dt.int32), w2[:, :, :])
                nc.vector.tensor_reduce(ot[:, 80:120], sel[:, :, :], axis=AxL.X, op=Alu.add)
                nc.vector.tensor_tensor(sel[:, :, :], maskm[:, :, :], bm(CD, RA),
                                        op=Alu.mult)
                nc.vector.tensor_reduce(ot[:, 120:160], sel[:, :, :], axis=AxL.X, op=Alu.add)
                # w0 = 1 - w1 - w2
                t40 = md.tile([P, RA], f32, name="t40", tag="t40")
                nc.vector.tensor_tensor(t40[:, :], ot[:, 40:80], ot[:, 80:120], op=Alu.add)
                nc.vector.tensor_scalar(ot[:, 0:40], t40[:, :], -1.0, 1.0,
                                        op0=Alu.mult, op1=Alu.add)
                # all-masked -> zero all four outputs
                allm = md.tile([P, RA], f32, name="allm", tag="allm")
                z40 = md.tile([P, RA], f32, name="z40", tag="z40")
                nc.vector.tensor_scalar(allm[:, :], m40[:, :], 1.0e29, None, op0=Alu.is_ge)
                nc.vector.memset(z40[:, :], 0.0)
                nc.vector.copy_predicated(ot[:, 0:40], allm[:, :].bitcast(mybir.dt.int32), z40[:, :])
                nc.vector.copy_predicated(ot[:, 40:80], allm[:, :].bitcast(mybir.dt.int32), z40[:, :])
                nc.vector.copy_predicated(ot[:, 80:120], allm[:, :].bitcast(mybir.dt.int32), z40[:, :])
                nc.vector.copy_predicated(ot[:, 120:160], allm[:, :].bitcast(mybir.dt.int32), z40[:, :])
                nc.sync.dma_start(out[b * P:(b + 1) * P, :], ot[:, :])
    nc.finalize()
    return nc


_NC = None


def _pack(template, projections):
    tm = np.asarray(template, np.float32).reshape(RA, 2)
    pr = np.asarray(projections, np.float32)
    A = pr[:, TRI[:, 0], :]
    B = pr[:, TRI[:, 1], :]
    C = pr[:, TRI[:, 2], :]
    code = (TRI[:, 0] + 8 * TRI[:, 1] + 64 * TRI[:, 2]).astype(np.float32)
    packed = np.empty((V, F_IN), np.float32)
    packed[:, oPX:oPX + NN] = pr[..., 0]
    packed[:, oPY:oPY + NN] = pr[..., 1]
    packed[:, oTX:oTX + RA] = tm[:, 0][None]
    packed[:, oTY:oTY + RA] = tm[:, 1][None]
    packed[:, oAX:oAX + T] = A[..., 0]
    packed[:, oAY:oAY + T] = A[..., 1]
    packed[:, oBX:oBX + T] = B[..., 0]
    packed[:, oBY:oBY + T] = B[..., 1]
    packed[:, oCX:oCX + T] = C[..., 0]
    packed[:, oCY:oCY + T] = C[..., 1]
    packed[:, oCD:oCD + T] = code[None]
    return packed


def _unpack(o):
    # o: (V, 160) f32
    w0 = o[:, 0:40]
    w1 = o[:, 40:80]
    w2 = o[:, 80:120]
    cd = np.rint(o[:, 120:160]).astype(np.int32)
    bc = np.stack([w0, w1, w2], axis=-1).reshape(V, 5, 8, 3).astype(np.float64)
    idx = np.stack([cd % 8, (cd // 8) % 8, cd // 64], axis=-1)
    idx = idx.reshape(V, 5, 8, 3).astype(np.int32)
    return bc, idx


def kernel(template, projections):
    global _NC
    from concourse.bass_utils import run_bass_kernel_spmd
    packed = _pack(template, projections)
    in_maps = []
    for c in range(NCORES):
        s = np.empty((VPAD, F_IN), np.float32)
        s[:VC] = packed[c * VC:(c + 1) * VC]
        s[VC:] = s[:1]
        in_maps.append({"x": s})
    if _NC is None:
        _NC = _build()
    res = run_bass_kernel_spmd(_NC, in_maps, core_ids=list(range(NCORES)))
    o = np.concatenate([res.results[c]["out"][:VC] for c in range(NCORES)], axis=0)
    return _unpack(o)


# revision 10
# speedup vs baseline: 1773.3669x; 1773.3669x over previous
import numpy as np
from itertools import combinations

V = 3000
NCORES = 8
VC = V // NCORES          # 375 vertices per core
P = 128
NB = 3                    # blocks of 128 partitions per core
VPAD = NB * P             # 384
T = 56                    # triangles = C(8,3)
RA = 40                   # template points (5*8)
NN = 8                    # neighbors
F_IN = 488
F_OUT = 160
BIG = 1.0e30

TRI = np.array(list(combinations(range(NN), 3)), dtype=np.int64)  # (56,3) lex

# packed input offsets
oPX, oPY, oTX, oTY = 0, 8, 16, 56
oAX, oAY, oBX, oBY, oCX, oCY, oCD = 96, 152, 208, 264, 320, 376, 432


def _runs():
    i_runs, ij_runs = [], []
    t = 0
    while t < T:
        i = TRI[t, 0]
        t0 = t
        while t < T and TRI[t, 0] == i:
            t += 1
        i_runs.append((int(i), t0, t - t0))
    t = 0
    while t < T:
        i, j = TRI[t, 0], TRI[t, 1]
        t0 = t
        while t < T and TRI[t, 0] == i and TRI[t, 1] == j:
            t += 1
        ij_runs.append((int(i), int(j), t0, t - t0))
    return i_runs, ij_runs


def _build():
    from concourse import bacc, tile
    import concourse.mybir as mybir

    f32 = mybir.dt.float32
    Alu = mybir.AluOpType
    ActF = mybir.ActivationFunctionType
    AxL = mybir.AxisListType

    nc = bacc.Bacc(None, target_bir_lowering=False)
    x = nc.dram_tensor("x", [VPAD, F_IN], f32, kind="ExternalInput")
    out = nc.dram_tensor("out", [VPAD, F_OUT], f32, kind="ExternalOutput")
    i_runs, ij_runs = _runs()

    def bt(ap, n):  # (128, m) / (128, a, b) -> broadcast new LAST dim of n
        return ap.unsqueeze(len(ap.shape)).broadcast_to([*ap.shape, n])

    def bm(ap, m):  # (128, n) -> (128, m, n)
        return ap.unsqueeze(1).broadcast_to([P, m, ap.shape[1]])

    with tile.TileContext(nc) as tc:
        with tc.tile_pool(name="io", bufs=2) as io, \
             tc.tile_pool(name="sm", bufs=1) as sm, \
             tc.tile_pool(name="md", bufs=1) as md, \
             tc.tile_pool(name="bg", bufs=1) as bg:
            for b in range(NB):
                xt = io.tile([P, F_IN], f32, name="xt", tag="xt")
                nc.sync.dma_start(xt[:, :], x[b * P:(b + 1) * P, :])
                PX = xt[:, oPX:oPX + NN]
                PY = xt[:, oPY:oPY + NN]
                TX = xt[:, oTX:oTX + RA]
                TY = xt[:, oTY:oTY + RA]
                AX = xt[:, oAX:oAX + T]
                AY = xt[:, oAY:oAY + T]
                BX = xt[:, oBX:oBX + T]
                BY = xt[:, oBY:oBY + T]
                CX = xt[:, oCX:oCX + T]
                CY = xt[:, oCY:oCY + T]
                CD = xt[:, oCD:oCD + T]

                def s56(tag):
                    return sm.tile([P, T], f32, name=tag, tag=tag)

                # ---- per-triangle (56) edge vectors & dots ----
                v0x, v0y, v1x, v1y = s56("v0x"), s56("v0y"), s56("v1x"), s56("v1y")
                nc.vector.tensor_tensor(v0x[:, :], CX, AX, op=Alu.subtract)
                nc.vector.tensor_tensor(v0y[:, :], CY, AY, op=Alu.subtract)
                nc.vector.tensor_tensor(v1x[:, :], BX, AX, op=Alu.subtract)
                nc.vector.tensor_tensor(v1y[:, :], BY, AY, op=Alu.subtract)
                ta, tb = s56("ta"), s56("tb")
                d00, d01, d11, den, rden, s_o = (s56("d00"), s56("d01"),
                                                 s56("d11"), s56("den"),
                                                 s56("rden"), s56("s_o"))
                nc.vector.tensor_tensor(ta[:, :], v0x[:, :], v0x[:, :], op=Alu.mult)
                nc.vector.tensor_tensor(tb[:, :], v0y[:, :], v0y[:, :], op=Alu.mult)
                nc.vector.tensor_tensor(d00[:, :], ta[:, :], tb[:, :], op=Alu.add)
                nc.vector.tensor_tensor(ta[:, :], v0x[:, :], v1x[:, :], op=Alu.mult)
                nc.vector.tensor_tensor(tb[:, :], v0y[:, :], v1y[:, :], op=Alu.mult)
                nc.vector.tensor_tensor(d01[:, :], ta[:, :], tb[:, :], op=Alu.add)
                nc.vector.tensor_tensor(ta[:, :], v1x[:, :], v1x[:, :], op=Alu.mult)
                nc.vector.tensor_tensor(tb[:, :], v1y[:, :], v1y[:, :], op=Alu.mult)
                nc.vector.tensor_tensor(d11[:, :], ta[:, :], tb[:, :], op=Alu.add)
                nc.vector.tensor_tensor(ta[:, :], d00[:, :], d11[:, :], op=Alu.mult)
                nc.vector.tensor_tensor(tb[:, :], d01[:, :], d01[:, :], op=Alu.mult)
                nc.vector.tensor_tensor(den[:, :], ta[:, :], tb[:, :], op=Alu.subtract)
                nc.vector.reciprocal(rden[:, :], den[:, :])
                nc.vector.tensor_scalar(rden[:, :], rden[:, :], 1.0e18, None, op0=Alu.min)
                nc.vector.tensor_scalar(rden[:, :], rden[:, :], -1.0e18, None, op0=Alu.max)
                # orientation s = cross(B-A, C-A) = v1x*v0y - v1y*v0x
                nc.vector.tensor_tensor(ta[:, :], v1x[:, :], v0y[:, :], op=Alu.mult)
                nc.vector.tensor_tensor(tb[:, :], v1y[:, :], v0x[:, :], op=Alu.mult)
                nc.vector.tensor_tensor(s_o[:, :], ta[:, :], tb[:, :], op=Alu.subtract)

                # ---- affine coefficients for w1, w2 ----
                # w2 = (d11*dot02 - d01*dot12)*rden = a2*Tx + b2*Ty + c2
                # w1 = (d00*dot12 - d01*dot02)*rden = a1*Tx + b1*Ty + c1
                a2, b2, c2 = s56("a2"), s56("b2"), s56("c2")
                a1, b1, c1 = s56("a1"), s56("b1"), s56("c1")
                nc.vector.tensor_tensor(ta[:, :], d11[:, :], v0x[:, :], op=Alu.mult)
                nc.vector.tensor_tensor(tb[:, :], d01[:, :], v1x[:, :], op=Alu.mult)
                nc.vector.tensor_tensor(a2[:, :], ta[:, :], tb[:, :], op=Alu.subtract)
                nc.vector.tensor_tensor(a2[:, :], a2[:, :], rden[:, :], op=Alu.mult)
                nc.vector.tensor_tensor(ta[:, :], d11[:, :], v0y[:, :], op=Alu.mult)
                nc.vector.tensor_tensor(tb[:, :], d01[:, :], v1y[:, :], op=Alu.mult)
                nc.vector.tensor_tensor(b2[:, :], ta[:, :], tb[:, :], op=Alu.subtract)
                nc.vector.tensor_tensor(b2[:, :], b2[:, :], rden[:, :], op=Alu.mult)
                nc.vector.tensor_tensor(ta[:, :], a2[:, :], AX, op=Alu.mult)
                nc.vector.tensor_tensor(tb[:, :], b2[:, :], AY, op=Alu.mult)
                nc.vector.scalar_tensor_tensor(c2[:, :], ta[:, :], -1.0, tb[:, :],
                                               op0=Alu.mult, op1=Alu.subtract)
                nc.vector.tensor_tensor(ta[:, :], d00[:, :], v1x[:, :], op=Alu.mult)
                nc.vector.tensor_tensor(tb[:, :], d01[:, :], v0x[:, :], op=Alu.mult)
                nc.vector.tensor_tensor(a1[:, :], ta[:, :], tb[:, :], op=Alu.subtract)
                nc.vector.tensor_tensor(a1[:, :], a1[:, :], rden[:, :], op=Alu.mult)
                nc.vector.tensor_tensor(ta[:, :], d00[:, :], v1y[:, :], op=Alu.mult)
                nc.vector.tensor_tensor(tb[:, :], d01[:, :], v0y[:, :], op=Alu.mult)
                nc.vector.tensor_tensor(b1[:, :], ta[:, :], tb[:, :], op=Alu.subtract)
                nc.vector.tensor_tensor(b1[:, :], b1[:, :], rden[:, :], op=Alu.mult)
                nc.vector.tensor_tensor(ta[:, :], a1[:, :], AX, op=Alu.mult)
                nc.vector.tensor_tensor(tb[:, :], b1[:, :], AY, op=Alu.mult)
                nc.vector.scalar_tensor_tensor(c1[:, :], ta[:, :], -1.0, tb[:, :],
                                               op0=Alu.mult, op1=Alu.subtract)

                # ---- incircle / Delaunay on gpsimd, grid (P, T, NN) ----
                def g8(tag):
                    return md.tile([P, T, NN], f32, name=tag, tag=tag)

                iax, iay, ibx, iby, icx, icy = (g8("iax"), g8("iay"), g8("ibx"),
                                                g8("iby"), g8("icx"), g8("icy"))
                PXb = bm(PX, T)
                PYb = bm(PY, T)
                nc.gpsimd.tensor_tensor(iax[:, :, :], bt(AX, NN), PXb, op=Alu.subtract)
                nc.gpsimd.tensor_tensor(iay[:, :, :], bt(AY, NN), PYb, op=Alu.subtract)
                nc.gpsimd.tensor_tensor(ibx[:, :, :], bt(BX, NN), PXb, op=Alu.subtract)
                nc.gpsimd.tensor_tensor(iby[:, :, :], bt(BY, NN), PYb, op=Alu.subtract)
                nc.gpsimd.tensor_tensor(icx[:, :, :], bt(CX, NN), PXb, op=Alu.subtract)
                nc.gpsimd.tensor_tensor(icy[:, :, :], bt(CY, NN), PYb, op=Alu.subtract)
                iaz, ibz, icz, g1, g2 = (g8("iaz"), g8("ibz"), g8("icz"),
                                         g8("g1"), g8("g2"))
                nc.gpsimd.tensor_tensor(g1[:, :, :], iax[:, :, :], iax[:, :, :], op=Alu.mult)
                nc.gpsimd.tensor_tensor(g2[:, :, :], iay[:, :, :], iay[:, :, :], op=Alu.mult)
                nc.gpsimd.tensor_tensor(iaz[:, :, :], g1[:, :, :], g2[:, :, :], op=Alu.add)
                nc.gpsimd.tensor_tensor(g1[:, :, :], ibx[:, :, :], ibx[:, :, :], op=Alu.mult)
                nc.gpsimd.tensor_tensor(g2[:, :, :], iby[:, :, :], iby[:, :, :], op=Alu.mult)
                nc.gpsimd.tensor_tensor(ibz[:, :, :], g1[:, :, :], g2[:, :, :], op=Alu.add)
                nc.gpsimd.tensor_tensor(g1[:, :, :], icx[:, :, :], icx[:, :, :], op=Alu.mult)
                nc.gpsimd.tensor_tensor(g2[:, :, :], icy[:, :, :], icy[:, :, :], op=Alu.mult)
                nc.gpsimd.tensor_tensor(icz[:, :, :], g1[:, :, :], g2[:, :, :], op=Alu.add)
                # D = iax*(iby*icz - ibz*icy) + iay*(ibz*icx - ibx*icz)
                #     + iaz*(ibx*icy - iby*icx)
                m1, m2, m3, Dd = g8("m1"), g8("m2"), g8("m3"), g8("Dd")
                nc.gpsimd.tensor_tensor(g1[:, :, :], iby[:, :, :], icz[:, :, :], op=Alu.mult)
                nc.gpsimd.tensor_tensor(g2[:, :, :], ibz[:, :, :], icy[:, :, :], op=Alu.mult)
                nc.gpsimd.tensor_tensor(m1[:, :, :], g1[:, :, :], g2[:, :, :], op=Alu.subtract)
                nc.gpsimd.tensor_tensor(g1[:, :, :], ibz[:, :, :], icx[:, :, :], op=Alu.mult)
                nc.gpsimd.tensor_tensor(g2[:, :, :], ibx[:, :, :], icz[:, :, :], op=Alu.mult)
                nc.gpsimd.tensor_tensor(m2[:, :, :], g1[:, :, :], g2[:, :, :], op=Alu.subtract)
                nc.gpsimd.tensor_tensor(g1[:, :, :], ibx[:, :, :], icy[:, :, :], op=Alu.mult)
                nc.gpsimd.tensor_tensor(g2[:, :, :], iby[:, :, :], icx[:, :, :], op=Alu.mult)
                nc.gpsimd.tensor_tensor(m3[:, :, :], g1[:, :, :], g2[:, :, :], op=Alu.subtract)
                nc.gpsimd.tensor_tensor(g1[:, :, :], iax[:, :, :], m1[:, :, :], op=Alu.mult)
                nc.gpsimd.tensor_tensor(g2[:, :, :], iay[:, :, :], m2[:, :, :], op=Alu.mult)
                nc.gpsimd.tensor_tensor(Dd[:, :, :], g1[:, :, :], g2[:, :, :], op=Alu.add)
                nc.gpsimd.tensor_tensor(g1[:, :, :], iaz[:, :, :], m3[:, :, :], op=Alu.mult)
                nc.gpsimd.tensor_tensor(Dd[:, :, :], Dd[:, :, :], g1[:, :, :], op=Alu.add)
                # violation iff s*D > 0
                nc.gpsimd.tensor_tensor(g1[:, :, :], Dd[:, :, :], bt(s_o[:, :], NN),
                                        op=Alu.mult)
                nc.gpsimd.tensor_scalar(g2[:, :, :], g1[:, :, :], 0.0, None,
                                        op0=Alu.is_gt)
                cnt, penD = s56("cnt"), s56("penD")
                nc.vector.tensor_reduce(cnt[:, :], g2[:, :, :], axis=AxL.X, op=Alu.add)
                nc.vector.tensor_scalar(penD[:, :], cnt[:, :], 0.0, 1.0e6,
                                        op0=Alu.is_gt, op1=Alu.mult)

                # ---- big grid (P, RA, T) ----
                def big3(tag):
                    return bg.tile([P, RA, T], f32, name=tag, tag=tag)

                w1, w2 = big3("w1"), big3("w2")
                u1, u2, bb = big3("u1"), big3("u2"), big3("bb")
                qa, qb, qc = big3("qa"), big3("qb"), big3("qc")
                e1, e2, e3, e4 = big3("e1"), big3("e2"), big3("e3"), big3("e4")
                dist, dm = big3("dist"), big3("dm")
                TXb = bt(TX, T)
                TYb = bt(TY, T)
                # w2 chain on DVE
                nc.vector.tensor_tensor(u1[:, :, :], bm(a2[:, :], RA), TXb, op=Alu.mult)
                nc.vector.tensor_tensor(u2[:, :, :], bm(b2[:, :], RA), TYb, op=Alu.mult)
                nc.vector.tensor_tensor(w2[:, :, :], u1[:, :, :], u2[:, :, :], op=Alu.add)
                nc.vector.tensor_tensor(w2[:, :, :], w2[:, :, :], bm(c2[:, :], RA), op=Alu.add)
                # w1 chain on Pool
                nc.gpsimd.tensor_tensor(e1[:, :, :], bm(a1[:, :], RA), TXb, op=Alu.mult)
                nc.gpsimd.tensor_tensor(e2[:, :, :], bm(b1[:, :], RA), TYb, op=Alu.mult)
                nc.gpsimd.tensor_tensor(w1[:, :, :], e1[:, :, :], e2[:, :, :], op=Alu.add)
                nc.gpsimd.tensor_tensor(w1[:, :, :], w1[:, :, :], bm(c1[:, :], RA), op=Alu.add)
                # distances: A on Pool, B/C on DVE, squares+sqrt on ACT
                nc.gpsimd.tensor_tensor(e1[:, :, :], TXb, bm(AX, RA), op=Alu.subtract)
                nc.gpsimd.tensor_tensor(e2[:, :, :], TYb, bm(AY, RA), op=Alu.subtract)
                nc.scalar.activation(e1[:, :, :], e1[:, :, :], func=ActF.Square)
                nc.scalar.activation(e2[:, :, :], e2[:, :, :], func=ActF.Square)
                nc.gpsimd.tensor_tensor(qa[:, :, :], e1[:, :, :], e2[:, :, :], op=Alu.add)
                nc.scalar.activation(qa[:, :, :], qa[:, :, :], func=ActF.Sqrt)
                nc.vector.tensor_tensor(e3[:, :, :], TXb, bm(BX, RA), op=Alu.subtract)
                nc.vector.tensor_tensor(e4[:, :, :], TYb, bm(BY, RA), op=Alu.subtract)
                nc.scalar.activation(e3[:, :, :], e3[:, :, :], func=ActF.Square)
                nc.scalar.activation(e4[:, :, :], e4[:, :, :], func=ActF.Square)
                nc.vector.tensor_tensor(qb[:, :, :], e3[:, :, :], e4[:, :, :], op=Alu.add)
                nc.scalar.activation(qb[:, :, :], qb[:, :, :], func=ActF.Sqrt)
                nc.vector.tensor_tensor(e3[:, :, :], TXb, bm(CX, RA), op=Alu.subtract)
                nc.vector.tensor_tensor(e4[:, :, :], TYb, bm(CY, RA), op=Alu.subtract)
                nc.scalar.activation(e3[:, :, :], e3[:, :, :], func=ActF.Square)
                nc.scalar.activation(e4[:, :, :], e4[:, :, :], func=ActF.Square)
                nc.vector.tensor_tensor(qc[:, :, :], e3[:, :, :], e4[:, :, :], op=Alu.add)
                nc.scalar.activation(qc[:, :, :], qc[:, :, :], func=ActF.Sqrt)
                nc.vector.tensor_tensor(dist[:, :, :], qa[:, :, :], qb[:, :, :], op=Alu.add)
                nc.vector.tensor_tensor(dist[:, :, :], dist[:, :, :], qc[:, :, :], op=Alu.add)
                # penalty mask: bad = (w1<=0) + (w2<=0) + (w1+w2>=1), pen 1e6 each
                nc.gpsimd.tensor_tensor(u1[:, :, :], w1[:, :, :], w2[:, :, :], op=Alu.add)
                nc.gpsimd.tensor_scalar(u2[:, :, :], w1[:, :, :], 0.0, None, op0=Alu.is_le)
                nc.gpsimd.tensor_scalar(bb[:, :, :], w2[:, :, :], 0.0, None, op0=Alu.is_le)
                nc.gpsimd.tensor_tensor(u2[:, :, :], u2[:, :, :], bb[:, :, :], op=Alu.add)
                nc.gpsimd.tensor_scalar(bb[:, :, :], u1[:, :, :], 1.0, None, op0=Alu.is_ge)
                nc.gpsimd.tensor_tensor(u2[:, :, :], u2[:, :, :], bb[:, :, :], op=Alu.add)
                nc.vector.scalar_tensor_tensor(dm[:, :, :], u2[:, :, :], 1.0e6,
                                               dist[:, :, :], op0=Alu.mult, op1=Alu.add)
                nc.gpsimd.tensor_tensor(dm[:, :, :], dm[:, :, :], bm(penD[:, :], RA),
                                        op=Alu.add)

                # ---- argmin & selection ----
                ot = io.tile([P, F_OUT], f32, name="ot", tag="ot")
                m40 = md.tile([P, RA], f32, name="m40", tag="m40")
                nc.vector.tensor_reduce(m40[:, :], dm[:, :, :], axis=AxL.X, op=Alu.min)
                maskm = dist  # reuse buffer
                nc.vector.tensor_tensor(maskm[:, :, :], dm[:, :, :], bt(m40[:, :], T),
                                        op=Alu.is_equal)
                sel = dm  # reuse buffer
                nc.vector.tensor_tensor(sel[:, :, :], maskm[:, :, :], w1[:, :, :], op=Alu.mult)
                nc.vector.tensor_reduce(ot[:, 40:80], sel[:, :, :], axis=AxL.X, op=Alu.add)
                nc.vector.tensor_tensor(sel[:, :, :], maskm[:, :, :], w2[:, :, :], op=Alu.mult)
                nc.vector.tensor_reduce(ot[:, 80:120], sel[:, :, :], axis=AxL.X, op=Alu.add)
                nc.vector.tensor_tensor(sel[:, :, :], maskm[:, :, :], bm(CD, RA),
                                        op=Alu.mult)
                nc.vector.tensor_reduce(ot[:, 120:160], sel[:, :, :], axis=AxL.X, op=Alu.add)
                # w0 = 1 - w1 - w2
                t40 = md.tile([P, RA], f32, name="t40", tag="t40")
                nc.vector.tensor_tensor(t40[:, :], ot[:, 40:80], ot[:, 80:120], op=Alu.add)
                nc.vector.tensor_scalar(ot[:, 0:40], t40[:, :], -1.0, 1.0,
                                        op0=Alu.mult, op1=Alu.add)
                # all-masked -> zero all four outputs
                allm = md.tile([P, RA], f32, name="allm", tag="allm")
                z40 = md.tile([P, RA], f32, name="z40", tag="z40")
                nc.vector.tensor_scalar(allm[:, :], m40[:, :], 1.0e5, None, op0=Alu.is_ge)
                nc.vector.memset(z40[:, :], 0.0)
                nc.vector.copy_predicated(ot[:, 0:40], allm[:, :].bitcast(mybir.dt.int32), z40[:, :])
                nc.vector.copy_predicated(ot[:, 40:80], allm[:, :].bitcast(mybir.dt.int32), z40[:, :])
                nc.vector.copy_predicated(ot[:, 80:120], allm[:, :].bitcast(mybir.dt.int32), z40[:, :])
                nc.vector.copy_predicated(ot[:, 120:160], allm[:, :].bitcast(mybir.dt.int32), z40[:, :])
                nc.sync.dma_start(out[b * P:(b + 1) * P, :], ot[:, :])
    nc.finalize()
    return nc


_NC = None


def _pack(template, projections):
    tm = np.asarray(template, np.float32).reshape(RA, 2)
    pr = np.asarray(projections, np.float32)
    A = pr[:, TRI[:, 0], :]
    B = pr[:, TRI[:, 1], :]
    C = pr[:, TRI[:, 2], :]
    code = (TRI[:, 0] + 8 * TRI[:, 1] + 64 * TRI[:, 2]).astype(np.float32)
    packed = np.empty((V, F_IN), np.float32)
    packed[:, oPX:oPX + NN] = pr[..., 0]
    packed[:, oPY:oPY + NN] = pr[..., 1]
    packed[:, oTX:oTX + RA] = tm[:, 0][None]
    packed[:, oTY:oTY + RA] = tm[:, 1][None]
    packed[:, oAX:oAX + T] = A[..., 0]
    packed[:, oAY:oAY + T] = A[..., 1]
    packed[:, oBX:oBX + T] = B[..., 0]
    packed[:, oBY:oBY + T] = B[..., 1]
    packed[:, oCX:oCX + T] = C[..., 0]
    packed[:, oCY:oCY + T] = C[..., 1]
    packed[:, oCD:oCD + T] = code[None]
    return packed


def _unpack(o):
    # o: (V, 160) f32
    w0 = o[:, 0:40]
    w1 = o[:, 40:80]
    w2 = o[:, 80:120]
    cd = np.rint(o[:, 120:160]).astype(np.int32)
    bc = np.stack([w0, w1, w2], axis=-1).reshape(V, 5, 8, 3).astype(np.float64)
    idx = np.stack([cd % 8, (cd // 8) % 8, cd // 64], axis=-1)
    idx = idx.reshape(V, 5, 8, 3).astype(np.int32)
    return bc, idx


def kernel(template, projections):
    global _NC
    from concourse.bass_utils import run_bass_kernel_spmd
    packed = _pack(template, projections)
    in_maps = []
    for c in range(NCORES):
        s = np.empty((VPAD, F_IN), np.float32)
        s[:VC] = packed[c * VC:(c + 1) * VC]
        s[VC:] = s[:1]
        in_maps.append({"x": s})
    if _NC is None:
        _NC = _build()
    res = run_bass_kernel_spmd(_NC, in_maps, core_ids=list(range(NCORES)))
    o = np.concatenate([res.results[c]["out"][:VC] for c in range(NCORES)], axis=0)
    return _unpack(o)
